# revision 2
# baseline (speedup 1.0000x reference)
"""AttentionMIL pooling kernel v3: v2 + software pipelining across chunks.

v2 read x once (fp16, +ones column), transposed on the PE, and built the
bag selector on chip -- minimal HBM traffic (33.7 MB/core at the ~55
GB/s real rate) -- but emitted each 512-row chunk's stages back-to-back.
Engine queues are strict FIFO, so every cross-engine dependency hop
(PE->DVE->PE->ACT->PE->ACT->PE->DVE->PE per chunk) cost a semaphore
round-trip bubble on the PE.

v3 software-pipelines the stages with per-stage chunk offsets so each
cross-engine edge has ~a full chunk iteration of slack:

  iteration c emits:
    PE:  T(c) transposes | H(c-1) hT matmuls | S(c-2) score | G(c-3) eT
         | U(c-4) pooling
    DVE: Ec(c-3) eT copy | L(c-3) selE build | C(c) xt copy
    ACT: Th(c-1) tanh | E(c-2) exp

Stage dataflow (chunk = 4 subtiles of 128 rows = 512 rows):
  T: 8x [128,128] PE transposes of x chunk -> xt_ps (PSUM, fp16)
  C: DVE copy xt_ps -> xt_sb
  H: hT[a,r] = sum_d W1[d,a] xT[d,r], W1 halves stationary -> h_ps
  Th: ACT tanh(h_ps + b1 bias AP) -> th (fp16)
  S: PE w2 [A,1] stationary -> s_ps [1, 512]
  E: ACT exp -> e_row [1, 512] fp16
  G: 4x K=1 PE matmuls (lhsT = e_row chunk [1,128], rhs = 1x1 one)
     -> et_ps [128, 4]
  Ec: DVE copy -> et (fp32, per-partition scalars)
  L: DVE fused tensor_scalar: selE[p,b] = (iota[p,b]==seg[p]) * e[p]
  U: PE pooling: lhsT = selE [128,64], rhs = x subtile [128,257]
     (ones column -> col 256 of u_ps accumulates the denominator)

Host: sum (U, den) partials over cores, pooled = U/den,
out = pooled @ Wh + bh.
"""

import numpy as np

import concourse.mybir as mybir
import concourse.tile as tile
from concourse import bacc
from concourse.bass_utils import run_bass_kernel_spmd

F16 = np.float16

N_CORES = 8
N_TOTAL = 524288
D = 256
DP = D + 1  # x padded with ones column
A = 128
B = 64  # num bags
P = 128  # SBUF partitions
R = N_TOTAL // N_CORES  # rows per core
T = R // P  # 512 subtiles of 128 rows per core
S = 64  # subtiles per super tile (DMA batch = 4.2 MiB, 32.9 KB/line)
SUPERS = T // S
CH = 4  # subtiles per chunk (512 rows = PSUM bank width)
TOTCH = T // CH  # 128 chunks per core

_NC_CACHE = {}


def build_nc(R=R, S=S, bufs=3, n_cores=N_CORES, debug=False):
    T = R // P
    SUPERS = T // S
    TOTCH = T // CH
    CPS = S // CH  # chunks per super
    dt = mybir.dt
    nc = bacc.Bacc("TRN2", target_bir_lowering=False, debug=debug, num_devices=n_cores)

    xn_d = nc.dram_tensor("xn", [R, DP], dt.float16, kind="ExternalInput")
    seg_d = nc.dram_tensor("seg", [P, T], dt.float32, kind="ExternalInput")
    w1_d = nc.dram_tensor("w1", [D, A], dt.float16, kind="ExternalInput")
    w2_d = nc.dram_tensor("w2", [A, 1], dt.float16, kind="ExternalInput")
    b1_d = nc.dram_tensor("b1", [A, 1], dt.float16, kind="ExternalInput")
    iota_d = nc.dram_tensor("iota", [P, B], dt.float16, kind="ExternalInput")
    ident_d = nc.dram_tensor("ident", [P, P], dt.float16, kind="ExternalInput")
    uout_d = nc.dram_tensor("uout", [B, DP], dt.float32, kind="ExternalOutput")

    xn_view = xn_d.ap().rearrange("(s p a) d -> s p (a d)", p=P, a=S)
    S4 = S // 4  # first super split into 4 quarter DMAs: compute starts sooner
    w1_view = w1_d.ap().rearrange("(h p) a -> p h a", p=P)

    with tile.TileContext(nc) as tc:
        with (
            tc.tile_pool(name="persist", bufs=1) as persist,
            tc.tile_pool(name="xn_pool", bufs=bufs) as xn_pool,
            tc.tile_pool(name="xt_pool", bufs=3) as xt_pool,
            tc.tile_pool(name="th_pool", bufs=3) as th_pool,
            tc.tile_pool(name="e_pool", bufs=3) as e_pool,
            tc.tile_pool(name="sel_pool", bufs=3) as sel_pool,
            tc.tile_pool(name="out_pool", bufs=1) as out_pool,
            tc.tile_pool(name="psum_u", bufs=1, space="PSUM") as psum_u,
            tc.tile_pool(name="psum_xt", bufs=2, space="PSUM") as psum_xt,
            tc.tile_pool(name="psum_h", bufs=2, space="PSUM") as psum_h,
            tc.tile_pool(name="psum_s", bufs=2, space="PSUM") as psum_s,
        ):
            w1_sb = persist.tile([P, 2, A], dt.float16)
            nc.sync.dma_start(out=w1_sb, in_=w1_view)
            w2_sb = persist.tile([A, 1], dt.float16)
            nc.sync.dma_start(out=w2_sb, in_=w2_d.ap())
            b1_sb = persist.tile([A, 1], dt.float16)
            nc.sync.dma_start(out=b1_sb, in_=b1_d.ap())
            iota_sb = persist.tile([P, B], dt.float16)
            nc.sync.dma_start(out=iota_sb, in_=iota_d.ap())
            id_sb = persist.tile([P, P], dt.float16)
            nc.sync.dma_start(out=id_sb, in_=ident_d.ap())
            seg_sb = persist.tile([P, T], dt.float32)
            nc.sync.dma_start(out=seg_sb, in_=seg_d.ap())

            u_ps = psum_u.tile([B, DP], dt.float32)

            xns = {}  # super idx -> xn tile
            xn0 = []  # super 0 quarter tiles
            for k in range(4):
                q = persist.tile([P, S4 * DP], dt.float16, name=f"xn0_{k}")
                nc.sync.dma_start(
                    out=q, in_=xn_view[0][:, k * S4 * DP : (k + 1) * S4 * DP]
                )
                xn0.append(q)
            st = {}  # chunk idx -> per-stage state

            def chunk_src(c):
                """(x tile, subtile offset within tile) for chunk c."""
                sidx = c // CPS
                cc = c % CPS
                if sidx == 0:
                    return xn0[cc * CH // S4], (cc * CH) % S4
                if sidx not in xns:
                    xn = xn_pool.tile([P, S * DP], dt.float16, name="xn")
                    nc.sync.dma_start(out=xn, in_=xn_view[sidx])
                    xns[sidx] = xn
                return xns[sidx], cc * CH

            def stage_T(c):
                xn, a0 = chunk_src(c)
                xt_ps = psum_xt.tile([P, 2, CH * P], dt.float16, name="xt_ps")
                for i in range(CH):
                    a = a0 + i
                    for h in range(2):
                        nc.tensor.transpose(
                            xt_ps[:, h, i * P : (i + 1) * P],
                            xn[:, a * DP + h * P : a * DP + (h + 1) * P],
                            id_sb,
                        )
                st[c] = {"xn": xn, "a0": a0, "xt_ps": xt_ps}

            def stage_C(c):
                d = st[c]
                xt_sb = xt_pool.tile([P, 2, CH * P], dt.float16, name="xt_sb")
                nc.vector.tensor_copy(xt_sb, d.pop("xt_ps"))
                d["xt_sb"] = xt_sb

            def stage_H(c):
                d = st[c]
                h_ps = psum_h.tile([A, CH * P], dt.float32, name="h_ps")
                xt_sb = d.pop("xt_sb")
                for h in range(2):
                    nc.tensor.matmul(
                        h_ps,
                        lhsT=w1_sb[:, h, :],
                        rhs=xt_sb[:, h, :],
                        start=(h == 0),
                        stop=(h == 1),
                    )
                d["h_ps"] = h_ps

            def stage_Th(c):
                d = st[c]
                th = th_pool.tile([A, CH * P], dt.float16, name="th")
                nc.scalar.activation(
                    th, d.pop("h_ps"), mybir.ActivationFunctionType.Tanh, bias=b1_sb
                )
                d["th"] = th

            def stage_S(c):
                d = st[c]
                s_ps = psum_s.tile([1, CH * P], dt.float32, name="s_ps")
                nc.tensor.matmul(
                    s_ps, lhsT=w2_sb, rhs=d.pop("th"), start=True, stop=True
                )
                d["s_ps"] = s_ps

            def stage_E(c):
                d = st[c]
                e_row = e_pool.tile([1, CH * P], dt.float16, name="e_row")
                nc.scalar.activation(
                    e_row, d.pop("s_ps"), mybir.ActivationFunctionType.Exp
                )
                d["e_row"] = e_row

            def stage_G(c):
                d = st[c]
                et_ps = psum_u.tile([P, CH], dt.float32, name="et_ps")
                e_row = d.pop("e_row")
                for j in range(CH):
                    nc.tensor.matmul(
                        et_ps[:, j : j + 1],
                        lhsT=e_row[:, j * P : (j + 1) * P],
                        rhs=id_sb[0:1, 0:1],
                        start=True,
                        stop=True,
                        skip_group_check=True,
                    )
                d["et_ps"] = et_ps

            def stage_EcL(c):
                d = st[c]
                et = e_pool.tile([P, CH], dt.float32, name="et")
                nc.vector.tensor_copy(et, d.pop("et_ps"))
                sel = sel_pool.tile([P, CH, B], dt.float16, name="sel")
                for j in range(CH):
                    t = c * CH + j
                    nc.vector.tensor_scalar(
                        out=sel[:, j, :],
                        in0=iota_sb,
                        scalar1=seg_sb[:, t : t + 1],
                        scalar2=et[:, j : j + 1],
                        op0=mybir.AluOpType.is_equal,
                        op1=mybir.AluOpType.mult,
                    )
                d["sel"] = sel

            def stage_U(c):
                d = st.pop(c)
                sel = d["sel"]
                xn = d["xn"]
                a0 = d["a0"]
                for j in range(CH):
                    t = c * CH + j
                    nc.tensor.matmul(
                        u_ps,
                        lhsT=sel[:, j, :],
                        rhs=xn[:, (a0 + j) * DP : (a0 + j + 1) * DP],
                        start=(t == 0),
                        stop=(t == T - 1),
                        skip_group_check=True,
                    )

            for c in range(TOTCH + 5):
                # DVE first: far-back stages whose deps are long satisfied
                if 4 <= c < TOTCH + 4:
                    stage_EcL(c - 4)
                # PE: current transposes, then progressively older stages
                if c < TOTCH:
                    stage_T(c)
                if 1 <= c < TOTCH + 1:
                    stage_H(c - 1)
                    stage_Th(c - 1)
                if 2 <= c < TOTCH + 2:
                    stage_S(c - 2)
                    stage_E(c - 2)
                if 3 <= c < TOTCH + 3:
                    stage_G(c - 3)
                if 5 <= c:
                    stage_U(c - 5)
                if c < TOTCH:
                    stage_C(c)

            u_sb = out_pool.tile([B, DP], dt.float32)
            nc.vector.tensor_copy(u_sb, u_ps)
            nc.sync.dma_start(out=uout_d.ap(), in_=u_sb)

    nc.compile()
    return nc


def _get_nc():
    if "v3" not in _NC_CACHE:
        _NC_CACHE["v3"] = build_nc()
    return _NC_CACHE["v3"]


def kernel(x, segment_ids, num_bags, W1, b1, w2, b2, Wh, bh):
    x = np.asarray(x)
    segment_ids = np.asarray(segment_ids)
    W1 = np.asarray(W1)
    b1 = np.asarray(b1)
    w2 = np.asarray(w2)
    Wh = np.asarray(Wh)
    bh = np.asarray(bh)
    num_bags = int(num_bags)
    assert x.shape == (N_TOTAL, D) and num_bags == B

    nc = _get_nc()

    xpad = np.empty((N_TOTAL, DP), dtype=F16)
    xpad[:, :D] = x.astype(F16)
    xpad[:, D] = 1.0
    w1_in = np.ascontiguousarray(W1.astype(F16))
    w2_in = np.ascontiguousarray(w2.astype(F16).reshape(A, 1))
    b1_in = np.ascontiguousarray(b1.astype(F16).reshape(A, 1))
    iota_in = np.broadcast_to(np.arange(B, dtype=F16), (P, B))
    ident_in = np.eye(P, dtype=F16)

    in_maps = []
    for c in range(N_CORES):
        sl = slice(c * R, (c + 1) * R)
        seg_c = np.ascontiguousarray(
            segment_ids[sl]
            .reshape(SUPERS, P, S)
            .transpose(1, 0, 2)
            .reshape(P, T)
            .astype(np.float32)
        )
        in_maps.append(
            {
                "xn": xpad[sl],
                "seg": seg_c,
                "w1": w1_in,
                "w2": w2_in,
                "b1": b1_in,
                "iota": iota_in,
                "ident": ident_in,
            }
        )

    res = run_bass_kernel_spmd(nc, in_maps, core_ids=list(range(N_CORES)))

    U = np.zeros((B, D), np.float64)
    den = np.zeros((B,), np.float64)
    for c in range(N_CORES):
        u = res.results[c]["uout"].astype(np.float64)
        U += u[:, :D]
        den += u[:, D]
    pooled = np.where(den[:, None] > 0, U / np.where(den == 0, 1, den)[:, None], 0.0)
    out = pooled @ Wh.astype(np.float64) + bh.astype(np.float64)
    return out.astype(np.float32)


# revision 5
# speedup vs baseline: 1.0400x; 1.0400x over previous
"""AttentionMIL pooling kernel v3: v2 + software pipelining across chunks.

v2 read x once (fp16, +ones column), transposed on the PE, and built the
bag selector on chip -- minimal HBM traffic (33.7 MB/core at the ~55
GB/s real rate) -- but emitted each 512-row chunk's stages back-to-back.
Engine queues are strict FIFO, so every cross-engine dependency hop
(PE->DVE->PE->ACT->PE->ACT->PE->DVE->PE per chunk) cost a semaphore
round-trip bubble on the PE.

v3 software-pipelines the stages with per-stage chunk offsets so each
cross-engine edge has ~a full chunk iteration of slack:

  iteration c emits:
    PE:  T(c) transposes | H(c-1) hT matmuls | S(c-2) score | G(c-3) eT
         | U(c-4) pooling
    DVE: Ec(c-3) eT copy | L(c-3) selE build | C(c) xt copy
    ACT: Th(c-1) tanh | E(c-2) exp

Stage dataflow (chunk = 4 subtiles of 128 rows = 512 rows):
  T: 8x [128,128] PE transposes of x chunk -> xt_ps (PSUM, fp16)
  C: DVE copy xt_ps -> xt_sb
  H: hT[a,r] = sum_d W1[d,a] xT[d,r], W1 halves stationary -> h_ps
  Th: ACT tanh(h_ps + b1 bias AP) -> th (fp16)
  S: PE w2 [A,1] stationary -> s_ps [1, 512]
  E: ACT exp -> e_row [1, 512] fp16
  G: 4x K=1 PE matmuls (lhsT = e_row chunk [1,128], rhs = 1x1 one)
     -> et_ps [128, 4]
  Ec: DVE copy -> et (fp32, per-partition scalars)
  L: DVE fused tensor_scalar: selE[p,b] = (iota[p,b]==seg[p]) * e[p]
  U: PE pooling: lhsT = selE [128,64], rhs = x subtile [128,257]
     (ones column -> col 256 of u_ps accumulates the denominator)

Host: sum (U, den) partials over cores, pooled = U/den,
out = pooled @ Wh + bh.
"""

import numpy as np

import concourse.mybir as mybir
import concourse.tile as tile
from concourse import bacc
from concourse.bass_utils import run_bass_kernel_spmd

F16 = np.float16

N_CORES = 8
N_TOTAL = 524288
D = 256
DP = D + 1  # x padded with ones column
A = 128
B = 64  # num bags
P = 128  # SBUF partitions
R = N_TOTAL // N_CORES  # rows per core
T = R // P  # 512 subtiles of 128 rows per core
S = 64  # subtiles per super tile (DMA batch = 4.2 MiB, 32.9 KB/line)
SUPERS = T // S
CH = 4  # subtiles per chunk (512 rows = PSUM bank width)
TOTCH = T // CH  # 128 chunks per core

_NC_CACHE = {}


def build_nc(R=R, S=S, bufs=4, n_cores=N_CORES, debug=False):
    T = R // P
    SUPERS = T // S
    TOTCH = T // CH
    CPS = S // CH  # chunks per super
    dt = mybir.dt
    nc = bacc.Bacc("TRN2", target_bir_lowering=False, debug=debug, num_devices=n_cores)

    xn_d = nc.dram_tensor("xn", [R, DP], dt.float16, kind="ExternalInput")
    seg_d = nc.dram_tensor("seg", [P, T], dt.float32, kind="ExternalInput")
    w1_d = nc.dram_tensor("w1", [D, A], dt.float16, kind="ExternalInput")
    w2_d = nc.dram_tensor("w2", [A, 1], dt.float16, kind="ExternalInput")
    b1_d = nc.dram_tensor("b1", [A, 1], dt.float16, kind="ExternalInput")
    iota_d = nc.dram_tensor("iota", [P, B], dt.float16, kind="ExternalInput")
    ident_d = nc.dram_tensor("ident", [P, P], dt.float16, kind="ExternalInput")
    uout_d = nc.dram_tensor("uout", [B, DP], dt.float32, kind="ExternalOutput")

    xn_view = xn_d.ap().rearrange("(s p a) d -> s p (a d)", p=P, a=S)
    S4 = S // 4  # first super split into 4 quarter DMAs: compute starts sooner
    w1_view = w1_d.ap().rearrange("(h p) a -> p h a", p=P)

    with tile.TileContext(nc) as tc:
        with (
            tc.tile_pool(name="persist", bufs=1) as persist,
            tc.tile_pool(name="xn_pool", bufs=bufs) as xn_pool,
            tc.tile_pool(name="xt_pool", bufs=3) as xt_pool,
            tc.tile_pool(name="th_pool", bufs=3) as th_pool,
            tc.tile_pool(name="e_pool", bufs=3) as e_pool,
            tc.tile_pool(name="sel_pool", bufs=3) as sel_pool,
            tc.tile_pool(name="out_pool", bufs=1) as out_pool,
            tc.tile_pool(name="psum_u", bufs=1, space="PSUM") as psum_u,
            tc.tile_pool(name="psum_xt", bufs=2, space="PSUM") as psum_xt,
            tc.tile_pool(name="psum_h", bufs=2, space="PSUM") as psum_h,
            tc.tile_pool(name="psum_s", bufs=2, space="PSUM") as psum_s,
        ):
            w1_sb = persist.tile([P, 2, A], dt.float16)
            nc.sync.dma_start(out=w1_sb, in_=w1_view)
            w2_sb = persist.tile([A, 1], dt.float16)
            nc.sync.dma_start(out=w2_sb, in_=w2_d.ap())
            b1_sb = persist.tile([A, 1], dt.float16)
            nc.sync.dma_start(out=b1_sb, in_=b1_d.ap())
            iota_sb = persist.tile([P, B], dt.float16)
            nc.sync.dma_start(out=iota_sb, in_=iota_d.ap())
            id_sb = persist.tile([P, P], dt.float16)
            nc.sync.dma_start(out=id_sb, in_=ident_d.ap())
            seg_sb = persist.tile([P, T], dt.float32)
            nc.sync.dma_start(out=seg_sb, in_=seg_d.ap())

            u_ps = psum_u.tile([B, DP], dt.float32)

            xns = {}  # super idx -> xn tile
            xn0 = []  # super 0 quarter tiles
            for k in range(4):
                q = persist.tile([P, S4 * DP], dt.float16, name=f"xn0_{k}")
                nc.sync.dma_start(
                    out=q, in_=xn_view[0][:, k * S4 * DP : (k + 1) * S4 * DP]
                )
                xn0.append(q)
            st = {}  # chunk idx -> per-stage state

            def chunk_src(c):
                """(x tile, subtile offset within tile) for chunk c."""
                sidx = c // CPS
                cc = c % CPS
                if sidx == 0:
                    return xn0[cc * CH // S4], (cc * CH) % S4
                if sidx not in xns:
                    xn = xn_pool.tile([P, S * DP], dt.float16, name="xn")
                    nc.sync.dma_start(out=xn, in_=xn_view[sidx])
                    xns[sidx] = xn
                return xns[sidx], cc * CH

            def stage_T(c):
                xn, a0 = chunk_src(c)
                xt_ps = psum_xt.tile([P, 2, CH * P], dt.float16, name="xt_ps")
                for i in range(CH):
                    a = a0 + i
                    for h in range(2):
                        nc.tensor.transpose(
                            xt_ps[:, h, i * P : (i + 1) * P],
                            xn[:, a * DP + h * P : a * DP + (h + 1) * P],
                            id_sb,
                        )
                st[c] = {"xn": xn, "a0": a0, "xt_ps": xt_ps}

            def stage_C(c):
                d = st[c]
                xt_sb = xt_pool.tile([P, 2, CH * P], dt.float16, name="xt_sb")
                nc.vector.tensor_copy(xt_sb, d.pop("xt_ps"))
                d["xt_sb"] = xt_sb

            def stage_H(c):
                d = st[c]
                h_ps = psum_h.tile([A, CH * P], dt.float32, name="h_ps")
                xt_sb = d.pop("xt_sb")
                for h in range(2):
                    nc.tensor.matmul(
                        h_ps,
                        lhsT=w1_sb[:, h, :],
                        rhs=xt_sb[:, h, :],
                        start=(h == 0),
                        stop=(h == 1),
                    )
                d["h_ps"] = h_ps

            def stage_Th(c):
                d = st[c]
                th = th_pool.tile([A, CH * P], dt.float16, name="th")
                nc.scalar.activation(
                    th, d.pop("h_ps"), mybir.ActivationFunctionType.Tanh, bias=b1_sb
                )
                d["th"] = th

            def stage_S(c):
                d = st[c]
                s_ps = psum_s.tile([1, CH * P], dt.float32, name="s_ps")
                nc.tensor.matmul(
                    s_ps, lhsT=w2_sb, rhs=d.pop("th"), start=True, stop=True
                )
                d["s_ps"] = s_ps

            def stage_E(c):
                d = st[c]
                e_row = e_pool.tile([1, CH * P], dt.float16, name="e_row")
                nc.scalar.activation(
                    e_row, d.pop("s_ps"), mybir.ActivationFunctionType.Exp
                )
                d["e_row"] = e_row

            def stage_G(c):
                d = st[c]
                et_ps = psum_u.tile([P, CH], dt.float32, name="et_ps")
                e_row = d.pop("e_row")
                for j in range(CH):
                    nc.tensor.matmul(
                        et_ps[:, j : j + 1],
                        lhsT=e_row[:, j * P : (j + 1) * P],
                        rhs=id_sb[0:1, 0:1],
                        start=True,
                        stop=True,
                        skip_group_check=True,
                    )
                d["et_ps"] = et_ps

            def stage_EcL(c):
                d = st[c]
                et = e_pool.tile([P, CH], dt.float32, name="et")
                nc.vector.tensor_copy(et, d.pop("et_ps"))
                sel = sel_pool.tile([P, CH, B], dt.float16, name="sel")
                for j in range(CH):
                    t = c * CH + j
                    nc.vector.tensor_scalar(
                        out=sel[:, j, :],
                        in0=iota_sb,
                        scalar1=seg_sb[:, t : t + 1],
                        scalar2=et[:, j : j + 1],
                        op0=mybir.AluOpType.is_equal,
                        op1=mybir.AluOpType.mult,
                    )
                d["sel"] = sel

            def stage_U(c):
                d = st.pop(c)
                sel = d["sel"]
                xn = d["xn"]
                a0 = d["a0"]
                for j in range(CH):
                    t = c * CH + j
                    nc.tensor.matmul(
                        u_ps,
                        lhsT=sel[:, j, :],
                        rhs=xn[:, (a0 + j) * DP : (a0 + j + 1) * DP],
                        start=(t == 0),
                        stop=(t == T - 1),
                        skip_group_check=True,
                    )

            for c in range(TOTCH + 5):
                # DVE first: far-back stages whose deps are long satisfied
                if 4 <= c < TOTCH + 4:
                    stage_EcL(c - 4)
                # PE: current transposes, then progressively older stages
                if c < TOTCH:
                    stage_T(c)
                if 1 <= c < TOTCH + 1:
                    stage_H(c - 1)
                    stage_Th(c - 1)
                if 2 <= c < TOTCH + 2:
                    stage_S(c - 2)
                    stage_E(c - 2)
                if 3 <= c < TOTCH + 3:
                    stage_G(c - 3)
                if 5 <= c:
                    stage_U(c - 5)
                if c < TOTCH:
                    stage_C(c)

            u_sb = out_pool.tile([B, DP], dt.float32)
            nc.vector.tensor_copy(u_sb, u_ps)
            nc.sync.dma_start(out=uout_d.ap(), in_=u_sb)

    nc.compile()
    return nc


def _get_nc():
    if "v3" not in _NC_CACHE:
        _NC_CACHE["v3"] = build_nc()
    return _NC_CACHE["v3"]


def kernel(x, segment_ids, num_bags, W1, b1, w2, b2, Wh, bh):
    x = np.asarray(x)
    segment_ids = np.asarray(segment_ids)
    W1 = np.asarray(W1)
    b1 = np.asarray(b1)
    w2 = np.asarray(w2)
    Wh = np.asarray(Wh)
    bh = np.asarray(bh)
    num_bags = int(num_bags)
    assert x.shape == (N_TOTAL, D) and num_bags == B

    nc = _get_nc()

    xpad = np.empty((N_TOTAL, DP), dtype=F16)
    xpad[:, :D] = x.astype(F16)
    xpad[:, D] = 1.0
    w1_in = np.ascontiguousarray(W1.astype(F16))
    w2_in = np.ascontiguousarray(w2.astype(F16).reshape(A, 1))
    b1_in = np.ascontiguousarray(b1.astype(F16).reshape(A, 1))
    iota_in = np.broadcast_to(np.arange(B, dtype=F16), (P, B))
    ident_in = np.eye(P, dtype=F16)

    in_maps = []
    for c in range(N_CORES):
        sl = slice(c * R, (c + 1) * R)
        seg_c = np.ascontiguousarray(
            segment_ids[sl]
            .reshape(SUPERS, P, S)
            .transpose(1, 0, 2)
            .reshape(P, T)
            .astype(np.float32)
        )
        in_maps.append(
            {
                "xn": xpad[sl],
                "seg": seg_c,
                "w1": w1_in,
                "w2": w2_in,
                "b1": b1_in,
                "iota": iota_in,
                "ident": ident_in,
            }
        )

    res = run_bass_kernel_spmd(nc, in_maps, core_ids=list(range(N_CORES)))

    U = np.zeros((B, D), np.float64)
    den = np.zeros((B,), np.float64)
    for c in range(N_CORES):
        u = res.results[c]["uout"].astype(np.float64)
        U += u[:, :D]
        den += u[:, D]
    pooled = np.where(den[:, None] > 0, U / np.where(den == 0, 1, den)[:, None], 0.0)
    out = pooled @ Wh.astype(np.float64) + bh.astype(np.float64)
    return out.astype(np.float32)
